# revision 34
# baseline (speedup 1.0000x reference)
"""KindredAttention on 8 trn2 NeuronCores.

Sharding: core(b, g) = b*2 + g for batch b in 0..3, head-group g in 0..1
(8 heads per group). Data-parallel over batch, tensor-parallel over heads
(qkv column-split, o_proj row-split; host sums the two o_proj partials).

Per-core layouts (host pre-transposes so the kernel never transposes):
  xt   [1024, 2048] bf16 : hidden[b].T                  (d-major)
  wqkv [1024, 1536] bf16 : qkv_w rows for this group, transposed.
                           cols = [q(8 heads x 64) | k(...) | v(...)]
  owt  [512, 1024]  bf16 : o_w[:, group cols].T
  cos/sina [128, 2048] bf16 : RoPE tables (2 heads stacked, sign-folded sin)
  out  [2048, 1024] f32  : partial o_proj output (host adds g=0 + g=1)

All matmuls bf16 (1 cycle/row at any HAM clock state; fp32r degrades 2x
when the PE is throttled).  Moving operands are 1024 wide where possible.
Attention: per (q-block 1024, head): S^T chunks [k128, q1024] -> exp on
ACT (scale 1/8 folded) -> PV with ones-augmented V (denominator free) ->
reciprocal_approx_fast + PE broadcast -> normalize -> o_proj per q-block
(overlaps attention tail).  PSUM drains ride the otherwise-idle ACT
engine in phase 1.
"""

import os

import ml_dtypes
import numpy as np

H = 16
D = 64
BASE = 10000.0
B, S, HD = 4, 2048, 1024
G = 2          # head groups (tensor parallel)
HG = H // G    # heads per group = 8
N_CORES = 8

last_results = None  # stash for test.py (exec_time_ns etc.)


def _rope_tables():
    inv_freq = 1.0 / (BASE ** (np.arange(0, D, 2, dtype=np.float32) / D))
    t = np.arange(S, dtype=np.float32)
    freqs = np.outer(t, inv_freq)                       # [S, 32]
    emb = np.concatenate([freqs, freqs], -1)            # [S, 64]
    cos = np.cos(emb).T.astype(np.float32)              # [64, S]
    sin = np.sin(emb).T.astype(np.float32)
    sina = sin.copy()
    sina[:32] = -sina[:32]                              # rotate_half sign fold
    # swap 32-row halves: row p holds the factor applied to source row p, so
    # the DVE rot-mul reads src and table at the SAME base partition
    # (SBUF-SBUF TensorTensor requires equal input base partitions).
    sins = np.concatenate([sina[32:], sina[:32]], axis=0)
    cos128 = np.tile(cos, (2, 1)).astype(ml_dtypes.bfloat16).copy()
    sins128 = np.tile(sins, (2, 1)).astype(ml_dtypes.bfloat16).copy()
    return cos128, sins128


def _build():
    import concourse.mybir as mybir
    import concourse.tile as tile
    from concourse import bacc
    from concourse.dve_ops import (
        RECIP_APPROX_FAST_CONSTS as RC,
        RECIPROCAL_APPROX_FAST,
    )

    F32 = mybir.dt.float32
    F32R = mybir.dt.float32r
    BF16 = mybir.dt.bfloat16
    Exp = mybir.ActivationFunctionType.Exp
    Copy = mybir.ActivationFunctionType.Copy

    nc = bacc.Bacc("TRN2", target_bir_lowering=False, debug=False,
                   num_devices=N_CORES)
    xt_d = nc.dram_tensor("xt", [HD, S], BF16, kind="ExternalInput")
    wq_d = nc.dram_tensor("wqkv", [HD, 3 * HG * D], BF16, kind="ExternalInput")
    ow_d = nc.dram_tensor("owt", [HG * D, HD], BF16, kind="ExternalInput")
    cos_d = nc.dram_tensor("cos", [128, S], BF16, kind="ExternalInput")
    sina_d = nc.dram_tensor("sina", [128, S], BF16, kind="ExternalInput")
    out_d = nc.dram_tensor("out", [S, HD], F32, kind="ExternalOutput")

    QB = S // 1024  # 2 big q/s blocks
    SC = S // 128   # 16 k-chunks
    DC = HD // 128  # 8 d-chunks

    with tile.TileContext(nc) as tc:
        with (
            tc.tile_pool(name="const", bufs=1) as constp,
            tc.tile_pool(name="persist", bufs=1) as persist,
        ):
            ones_f = constp.tile([1, 64], F32, tag="onesf")
            ones_sb = constp.tile([1, 64], F32R, tag="ones")
            nc.vector.memset(ones_f[:], 1.0)
            nc.vector.tensor_copy(ones_sb[:], ones_f[:])

            # q rows: fc 0-3, k rows: fc 4-7 (feature-major, 2 heads/tile)
            qk_sb = persist.tile([128, 8, S], BF16, tag="qk")
            v_sb = persist.tile([128, SC, HG, D + 1], BF16, tag="v")
            nc.vector.memset(v_sb[:], 1.0)  # ones column survives at [..., 64]

            # ---------------- phase 1: qkv projection + RoPE ----------------
            with (
                tc.tile_pool(name="w1", bufs=1) as w1p,
                tc.tile_pool(name="xts", bufs=2) as xtp,
                tc.tile_pool(name="ps1", bufs=2, space="PSUM") as ps1,
                tc.tile_pool(name="psv", bufs=2, space="PSUM") as psv,
                tc.tile_pool(name="rope", bufs=2) as ropep,
            ):
                cos_sb = w1p.tile([128, S], BF16, tag="cos")
                sina_sb = w1p.tile([128, S], BF16, tag="sina")
                wq_sb = w1p.tile([128, DC, 3 * HG * D], BF16, tag="wq")
                nc.sync.dma_start(cos_sb[:], cos_d[:])
                nc.sync.dma_start(sina_sb[:], sina_d[:])
                nc.sync.dma_start(
                    wq_sb[:], wq_d[:].rearrange("(a p) f -> p a f", p=128)
                )
                for sb in range(QB):
                    ssl = slice(sb * 1024, (sb + 1) * 1024)
                    xts = xtp.tile([128, DC, 1024], BF16, tag="xts")
                    nc.sync.dma_start(
                        xts[:],
                        xt_d[:].rearrange("(a p) s -> p a s", p=128)[:, :, ssl],
                    )
                    for fc in range(8):  # q chunks 0-3, k chunks 4-7
                        ps = ps1.tile([128, 1024], F32, tag="ps1")
                        for dc in range(DC):  # dc outer: stationary reused j=0,1
                            for j in range(2):
                                jsl = slice(j * 512, (j + 1) * 512)
                                nc.tensor.matmul(
                                    ps[:, jsl],
                                    wq_sb[:, dc, fc * 128:(fc + 1) * 128],
                                    xts[:, dc, jsl],
                                    start=(dc == 0), stop=(dc == DC - 1),
                                )
                        # drain to bf16 on ACT (idle in phase 1)
                        t0 = ropep.tile([128, 1024], BF16, tag="t0")
                        nc.scalar.activation(t0[:], ps[:], Copy)
                        t1 = ropep.tile([128, 1024], BF16, tag="t1")
                        for r in (0, 64):  # two heads per chunk
                            nc.vector.tensor_mul(
                                t1[r:r + 32, :], t0[r + 32:r + 64, :],
                                sina_sb[r + 32:r + 64, ssl])
                            nc.vector.tensor_mul(
                                t1[r + 32:r + 64, :], t0[r:r + 32, :],
                                sina_sb[r:r + 32, ssl])
                        m1 = ropep.tile([128, 1024], BF16, tag="m1")
                        nc.vector.tensor_mul(m1[:], t0[:], cos_sb[:, ssl])
                        nc.vector.tensor_add(qk_sb[:, fc, ssl], m1[:], t1[:])
                    for s4 in range(8):  # v s-chunks in this block
                        sc = sb * 8 + s4
                        ps = psv.tile([128, 512], F32, tag="psv")
                        for dc in range(DC):
                            nc.tensor.matmul(
                                ps[:],
                                xts[:, dc, s4 * 128:(s4 + 1) * 128],
                                wq_sb[:, dc, 1024:1536],
                                start=(dc == 0), stop=(dc == DC - 1),
                            )
                        nc.scalar.activation(
                            v_sb[:, sc, :, 0:D],
                            ps[:].rearrange("p (h d) -> p h d", d=D),
                            Copy)

            # ---------------- phase 2+3: attention + o_proj ----------------
            # exp split: even k-chunks on ACT (LUT exp), odd on DVE via a
            # one-op Schraudolph directly in bf16 bit space:
            #   bits16 = round(s * 16/ln2 + C)  ->  bitcast bf16 ~= exp(s/8)
            EXPA = 0.125 * 128.0 / float(np.log(2.0))
            EXPC = 16248.60
            I16 = mybir.dt.int16
            Mult = mybir.AluOpType.mult
            Add = mybir.AluOpType.add
            with (
                tc.tile_pool(name="persist2", bufs=1) as persist2,
                tc.tile_pool(name="psqk", bufs=2, space="PSUM") as psqk,
                tc.tile_pool(name="pspv", bufs=1, space="PSUM") as pspv,
                tc.tile_pool(name="psbc", bufs=1, space="PSUM") as psbc,
                tc.tile_pool(name="es", bufs=4) as esp,
                tc.tile_pool(name="rcp", bufs=2) as rcp,
                tc.tile_pool(name="ot", bufs=2) as otp,
                tc.tile_pool(name="og", bufs=3) as ogp,
            ):
                ow_sb = persist2.tile([128, 4, HD], BF16, tag="ow")
                nc.sync.dma_start(
                    ow_sb[:], ow_d[:].rearrange("(a p) f -> p a f", p=128)
                )
                for qb in range(QB):
                    qsl = slice(qb * 1024, (qb + 1) * 1024)
                    ot = otp.tile([128, 4, 1024], BF16, tag="ot")
                    for h in range(HG):
                        hp = (h % 2) * 64
                        pv = pspv.tile([128, 1024], F32, tag="pv")
                        es_t = [None] * SC
                        # software-pipelined: PV runs two chunks behind scores
                        for ci in range(SC + 2):
                            if ci < SC:
                                c = ci
                                qs = psqk.tile([128, 1024], F32, tag="qs")
                                for j in range(2):
                                    jsl = slice(j * 512, (j + 1) * 512)
                                    qj = slice(qb * 1024 + j * 512,
                                               qb * 1024 + (j + 1) * 512)
                                    nc.tensor.matmul(
                                        qs[:, jsl],
                                        qk_sb[hp:hp + 64, 4 + h // 2,
                                              c * 128:(c + 1) * 128],
                                        qk_sb[hp:hp + 64, h // 2, qj],
                                        start=True, stop=True,
                                    )
                                es = esp.tile([128, 1024], BF16, tag="es")
                                if c % 2 == 0:
                                    nc.scalar.activation(
                                        es[:], qs[:], Exp, scale=0.125)
                                else:
                                    with nc.allow_low_precision(
                                            reason="schraudolph exp bf16"):
                                        nc.vector.tensor_scalar(
                                            es[:].bitcast(I16), qs[:],
                                            EXPA, EXPC, Mult, Add)
                                es_t[c] = es
                            if ci >= 2:
                                c = ci - 2
                                for j in range(2):
                                    jsl = slice(j * 512, (j + 1) * 512)
                                    nc.tensor.matmul(
                                        pv[0:D + 1, jsl],
                                        v_sb[:, c, h, :],
                                        es_t[c][:, jsl],
                                        start=(c == 0), stop=(c == SC - 1),
                                    )
                        # stage denom row 64 to SBUF on ACT (off the DVE
                        # chain); the custom-DVE recip needs a base-partition
                        # AP (it mis-reads nonzero partition offsets)
                        den = rcp.tile([1, 1024], F32, tag="den")
                        nc.scalar.activation(den[:], pv[D:D + 1, :], Copy)
                        rc = rcp.tile([1, 1024], F32R, tag="rc")
                        with nc.allow_low_precision(reason="softmax denom recip"):
                            nc.vector._custom_dve(
                                RECIPROCAL_APPROX_FAST, out=rc[:], in0=den[:],
                                s0=RC["s0"], s1=RC["s1"], imm2=RC["imm2"])
                        bcs = rcp.tile([64, 1024], F32, tag="bcs")
                        for j in range(2):
                            bc = psbc.tile([128, 512], F32, tag="bc")
                            nc.tensor.matmul(
                                bc[0:64, :], ones_sb[:],
                                rc[:, j * 512:(j + 1) * 512],
                                start=True, stop=True)
                            nc.vector.tensor_copy(
                                bcs[:, j * 512:(j + 1) * 512], bc[0:64, :])
                        nc.vector.tensor_mul(
                            ot[hp:hp + 64, h // 2, :], pv[0:D, :], bcs[:])
                    # o_proj for this q-block (overlaps next block's attention;
                    # po shares the psqk PSUM buffers via the same tag)
                    for scq in range(8):
                        po = psqk.tile([128, 1024], F32, tag="qs")
                        for j in range(2):
                            jsl = slice(j * 512, (j + 1) * 512)
                            for oc in range(4):
                                nc.tensor.matmul(
                                    po[:, jsl],
                                    ot[:, oc, scq * 128:(scq + 1) * 128],
                                    ow_sb[:, oc, jsl],
                                    start=(oc == 0), stop=(oc == 3),
                                )
                        og = ogp.tile([128, HD], F32, tag="og")
                        nc.scalar.activation(og[:], po[:], Copy)
                        nc.sync.dma_start(
                            out_d[qb * 1024 + scq * 128:
                                  qb * 1024 + (scq + 1) * 128, :], og[:])

    nc.compile()
    return nc


def kernel(hidden_states, qkv_w, o_w):
    global last_results
    from concourse.bass_utils import run_bass_kernel_spmd

    hidden_states = np.asarray(hidden_states, dtype=np.float32)
    qkv_w = np.asarray(qkv_w, dtype=np.float32)
    o_w = np.asarray(o_w, dtype=np.float32)

    cos128, sina128 = _rope_tables()
    nc = _build()

    in_maps = []
    for core in range(N_CORES):
        b, g = core // G, core % G
        heads = range(g * HG, (g + 1) * HG)
        rows = np.concatenate(
              [np.arange(h * D, (h + 1) * D) for h in heads])
        wsel = np.concatenate(
              [qkv_w[off + rows] for off in (0, HD, 2 * HD)], axis=0)  # [1536,1024]
        in_maps.append({
              "xt": np.ascontiguousarray(hidden_states[b].T).astype(
                  ml_dtypes.bfloat16),
              "wqkv": np.ascontiguousarray(wsel.T).astype(ml_dtypes.bfloat16),
              "owt": np.ascontiguousarray(o_w[:, rows].T).astype(
                  ml_dtypes.bfloat16),
              "cos": cos128,
              "sina": sina128,
        })

    trace = bool(int(os.environ.get("KERNEL_TRACE", "0")))
    try:
        last_results = run_bass_kernel_spmd(
            nc, in_maps, core_ids=list(range(N_CORES)), trace=trace)
    except ModuleNotFoundError:
        # axon NTFF hook unavailable in this container; run without trace
        last_results = run_bass_kernel_spmd(
            nc, in_maps, core_ids=list(range(N_CORES)), trace=False)

    out = np.empty((B, S, HD), dtype=np.float32)
    for b in range(B):
        out[b] = last_results.results[b * G]["out"]
        for g in range(1, G):
              out[b] += last_results.results[b * G + g]["out"]
    return out
